# revision 16
# baseline (speedup 1.0000x reference)
"""Binary conv1d block (RSign -> BinaryConv1d(k=3, pad=1) -> bias -> RPReLU).

Strategy (8 NeuronCores, data-parallel over batch):
  - Each core gets 2 of the 16 batch images; params replicated.
  - Partition packing p = b*64 + ch puts both images' 64 channels in the
    128 SBUF partitions, so every elementwise op runs at full lane width.
  - RSign: one DVE tensor_scalar is_ge producing g = (x >= alpha) in {1,0}
    as bf16 (exact).  The conv over xb = 2g-1 is recovered affinely:
    y = 2*sum(wb*g) - c + b with per-channel c = sum(wb); the zero-padded
    boundary columns use corrected constants c0/cL.
  - Conv as 3 accumulated 128x128 matmuls per 512-col psum chunk with
    block-diagonal (per-image) +/-1 weights in bf16 -- integer-exact.
  - RPReLU: out = max(beta*y,0) - max(-gamma*y,0) + zeta (beta,gamma >= 0).
    Both relu terms are single ScalarE activations with per-partition
    scale/bias APs reading PSUM; the combine (+zeta fold) is one DVE
    scalar_tensor_tensor.
"""

import os
import sys

import numpy as np

for _p in (
    "/root/.axon_site",
    "/root/.axon_site/_ro/trn_rl_repo",
    "/root/.axon_site/_ro/pypackages",
    "/opt/trn_rl_repo",
    "/opt/pypackages",
):
    if os.path.isdir(_p) and _p not in sys.path:
        sys.path.append(_p)

import ml_dtypes

import bass_rust
import concourse.bass as bass
import concourse.tile as tile
from concourse import bacc
from concourse import mybir
from concourse.bass_utils import run_bass_kernel_spmd

B, CIN, COUT, K, L = 16, 64, 64, 3, 65536
NCORES = 8
BPC = B // NCORES  # images per core
P = 128  # partitions = BPC * 64 channels
LT = 2048  # output columns per tile
NT = L // LT
MMN = 512  # matmul free dim (one fp32 PSUM bank)
NCHUNK = LT // MMN

F32 = mybir.dt.float32
BF16 = mybir.dt.bfloat16
F16 = mybir.dt.float16
FP8 = mybir.dt.float8e4

LAST_RESULTS = None
_NC_CACHE = {}


def _build_nc_v3(lt=LT, nt=NT, l_total=L, repeat=1, dma_only=False, taper=True,
                 ratio=0.2, nxb=4):
    """v3: fp8 DoubleRow conv with all scales/bias folded into the weights.

    - ACT Sign(x + (eps - alpha)) -> xb in {+-1} fp8 (plane 0 of a manual
      ring buffer whose plane 1 is constant 1.0).  The zero-padded sequence
      edge is a memset halo column -- exact, no boundary corrections.
    - PE: 2 DoubleRow fp8 matmuls per 512-col chunk: taps (0,1) via an
      overlapping [128][2][512] AP (subtile stride 1 col), taps (2, bias)
      via the natural plane-pair slice; the bias subtile reads the ones
      plane, its weights hold beta*b per channel.  Weights are beta*wb
      (exact in fp8 when beta is), so PSUM = beta*y directly.
    - Epilogue: ACT Copy psum->u (f16), DVE stt out = max(ratio*u, psum)
      = max(gamma*y, beta*y) which equals RPReLU for beta >= gamma >= 0,
      zeta == 0.  Output stored fp16 (halves store-side HBM traffic).
    """
    nc = bacc.Bacc()
    x_d = nc.dram_tensor("x", [BPC, CIN, l_total], F32, kind="ExternalInput")
    w_d = nc.dram_tensor("wq", [P, 2, 2, P], FP8, kind="ExternalInput")
    c_d = nc.dram_tensor("consts", [P, 1], F32, kind="ExternalInput")
    o_d = nc.dram_tensor("out", [BPC, COUT, l_total], F16, kind="ExternalOutput")

    x_f = x_d[:].rearrange("b c l -> (b c) l")
    o_f = o_d[:].rearrange("b c l -> (b c) l")

    Sign = mybir.ActivationFunctionType.Sign
    Copy = mybir.ActivationFunctionType.Copy
    Op = mybir.AluOpType
    DR = mybir.MatmulPerfMode.DoubleRow
    WB = lt + 4  # ring-slot width: col0 left halo, 1..w+1 data, pad

    load_dma = nc.sync.dma_start
    store_dma = nc.gpsimd.dma_start

    with tile.TileContext(nc) as tc:
        with (
            tc.tile_pool(name="const", bufs=1) as constp,
            tc.tile_pool(name="xin", bufs=7) as xin,
            tc.tile_pool(name="up", bufs=3) as up,
            tc.tile_pool(name="outp", bufs=7) as outp,
            tc.tile_pool(name="ps", bufs=max(1, 4096 // lt), space="PSUM") as psp,
        ):
            wt = constp.tile([P, 2, 2, P], FP8)
            nc.sync.dma_start(wt[:], w_d[:])
            ct = constp.tile([P, 1], F32)
            nc.sync.dma_start(ct[:], c_d[:])
            sbias = ct[:, 0:1]  # eps - alpha per partition
            xbuf = constp.tile([P, 2, nxb, WB], FP8)
            nc.vector.memset(xbuf[:, 1, :, :], 1.0)  # ones plane (bias tap)
            pstr = xbuf[:].ap[0][0]

            def sign(dst, src):
                nc.scalar.activation(dst, src, Sign, bias=sbias, scale=1.0)

            if taper and not dma_only:
                tiles_per_pass = [(i * lt, lt) for i in range(nt - 1)] + [
                    (l_total - lt + i * MMN, MMN) for i in range(lt // MMN)
                ]
            else:
                tiles_per_pass = [(i * lt, lt) for i in range(nt)]

            def emit_compute(bidx, s_, w_, last):
                o_t = outp.tile([P, w_], F16, tag="o_t")
                xb0 = xbuf[:, 0, bidx, :]
                for co in range(0, w_, lt):
                    cw = min(lt, w_ - co)
                    ps = psp.tile([P, cw], F32, tag="ps")
                    for j in range(cw // MMN):
                        bc = co + j * MMN
                        rhs1 = xb0[:, bc : bc + MMN].copy()
                        rhs1.ap = bass_rust.VecI64Pair(
                            [[pstr, P], [1, 2], [1, MMN]]
                        )
                        nc.tensor.matmul(
                            ps[:, j * MMN : (j + 1) * MMN],
                            wt[:, 0, :, :], rhs1,
                            start=True, stop=False, perf_mode=DR,
                        )
                        rhs2 = xbuf[:, :, bidx, bc + 2 : bc + 2 + MMN]
                        nc.tensor.matmul(
                            ps[:, j * MMN : (j + 1) * MMN],
                            wt[:, 1, :, :], rhs2,
                            start=False, stop=True, perf_mode=DR,
                        )
                    u = up.tile([P, cw], F16, tag="u")
                    nc.scalar.activation(u[:], ps[:], Copy, bias=0.0, scale=1.0)
                    nc.vector.scalar_tensor_tensor(
                        o_t[:, co : co + cw], u[:], ratio, ps[:], Op.mult, Op.max
                    )
                store_dma(o_f[:, s_ : s_ + w_], o_t[:])

            prev_x = None
            prev_w = 0
            pend = None  # (bidx, s_, w_) awaiting right halo + compute
            dummy = None
            if dma_only:
                dummy = constp.tile([P, lt], F16)
                nc.vector.memset(dummy[:], 0.0)
            for idx, (s_, w_) in enumerate(
                [tw for _ in range(repeat) for tw in tiles_per_pass]
            ):
                first = s_ == 0
                last = s_ + w_ == l_total
                x_t = xin.tile([P, w_], F32, tag="x_t")
                if dma_only:
                    load_dma(x_t[:], x_f[:, s_ : s_ + w_])
                    store_dma(o_f[:, s_ : s_ + w_], dummy[:, 0:w_])
                    continue
                load_dma(x_t[:], x_f[:, s_ : s_ + w_])
                if pend is not None:
                    pb, ps_, pw_ = pend
                    plast = ps_ + pw_ == l_total
                    pxb = xbuf[:, 0, pb, :]
                    if plast:
                        nc.vector.memset(pxb[:, pw_ + 1 : pw_ + 2], 0.0)
                    else:
                        sign(pxb[:, pw_ + 1 : pw_ + 2], x_t[:, 0:1])
                    emit_compute(pb, ps_, pw_, plast)
                bidx = idx % nxb
                xb0 = xbuf[:, 0, bidx, :]
                if first:
                    nc.vector.memset(xb0[:, 0:1], 0.0)
                else:
                    sign(xb0[:, 0:1], prev_x[:, prev_w - 1 : prev_w])
                sign(xb0[:, 1 : w_ + 1], x_t[:])
                prev_x, prev_w = x_t, w_
                pend = (bidx, s_, w_)

            if pend is not None and not dma_only:
                pb, ps_, pw_ = pend
                nc.vector.memset(xbuf[:, 0, pb, pw_ + 1 : pw_ + 2], 0.0)
                emit_compute(pb, ps_, pw_, True)
    nc.compile()
    return nc


def build_nc(**kw):
    kw = dict(kw)
    if kw.pop("v3", False):
        kw.pop("alpha_imm", None)
        kw.pop("zeta_imm", None)
        kw.pop("lrelu", None)
        kw.pop("out_f16", None)
        return _build_nc_v3(**kw)
    kw.pop("ratio", None)
    kw.pop("nxb", None)
    return _build_nc(**kw)


def _build_nc(lt=LT, nt=NT, l_total=L, alpha_imm=0.0, zeta_imm=0.0, repeat=1, dt_mult=1, dma_only=False, taper=True, lrelu=False, out_f16=False):
    """alpha_imm/zeta_imm: float immediates when those params are
    channel-uniform (walrus rejects TensorScalarPtr with >1 sync wait);
    None selects the general per-partition path.
    repeat: unroll the whole pipeline R times (benchmarking only).
    dt_mult: DMA tile width = dt_mult * lt (compute chunks stay lt wide).
    lrelu: single-ACT epilogue out = Lrelu(2b*ps + b(bias-c); slope g/b)
    (valid when zeta == 0, beta > 0); replaces 2 ACT + 1 DVE combine.
    out_f16: store the output as fp16 (halves store-side HBM traffic;
    host upcasts).  Exact conv integers are <= 192 so fp16 rel err
    <= 2^-11, far inside the 2e-2 gate."""
    nc = bacc.Bacc()
    ODT = F16 if out_f16 else F32
    x_d = nc.dram_tensor("x", [BPC, CIN, l_total], F32, kind="ExternalInput")
    w_d = nc.dram_tensor("wmats", [P, K * P], BF16, kind="ExternalInput")
    c_d = nc.dram_tensor("consts", [P, 11], F32, kind="ExternalInput")
    o_d = nc.dram_tensor("out", [BPC, COUT, l_total], ODT, kind="ExternalOutput")

    x_f = x_d[:].rearrange("b c l -> (b c) l")
    o_f = o_d[:].rearrange("b c l -> (b c) l")

    Relu = mybir.ActivationFunctionType.Relu
    Lrelu = mybir.ActivationFunctionType.Lrelu
    Op = mybir.AluOpType
    nchunk = lt // MMN
    W = lt * dt_mult  # DMA tile width
    assert nt % dt_mult == 0
    nd = nt // dt_mult
    assert nd >= 2, "need at least 2 DMA tiles (separate first/last edges)"
    xin_bufs = 7 if dt_mult == 1 else 3
    out_bufs = 7 if dt_mult == 1 else 3
    g_bufs = 4 if dt_mult == 1 else 2

    # alpha == 0: bf16(x) >= 0 iff x >= 0 (rounding never crosses zero), so
    # the load can cast f32->bf16 in the SDMA datapath, halving the
    # SBUF-port write traffic.  Cast DMAs are SWDGE-only, so loads move to
    # gpsimd and stores take the now-uncontended sync HWDGE ring.
    # Measured on HW: the cast-load saves SBUF-port bytes but not time --
    # the HBM read side (f32 bytes unchanged) is the binder, and SWDGE load
    # dispatch adds overhead.  Keep the HWDGE f32-load path.
    cast_load = False
    xdt = BF16 if cast_load else F32
    load_dma = nc.gpsimd.dma_start if cast_load else nc.sync.dma_start
    store_dma = nc.sync.dma_start if cast_load else nc.gpsimd.dma_start

    with tile.TileContext(nc) as tc:
        with (
            tc.tile_pool(name="const", bufs=1) as constp,
            tc.tile_pool(name="xin", bufs=xin_bufs) as xin,
            tc.tile_pool(name="gbuf", bufs=g_bufs) as gbuf,
            tc.tile_pool(name="am", bufs=4) as am,
            tc.tile_pool(name="outp", bufs=out_bufs) as outp,
            tc.tile_pool(name="fix", bufs=2) as fixp,
            tc.tile_pool(name="ps", bufs=max(1, 4096 // lt), space="PSUM") as psp,
        ):
            wt = constp.tile([P, K * P], BF16)
            nc.sync.dma_start(wt[:], w_d[:])
            ct = constp.tile([P, 11], F32)
            nc.sync.dma_start(ct[:], c_d[:])
            alpha = ct[:, 0:1]
            scA = ct[:, 1:2]
            bA = ct[:, 2:3]
            scM = ct[:, 3:4]
            bM = ct[:, 4:5]
            zeta = ct[:, 5:6]
            bA0 = ct[:, 6:7]
            bM0 = ct[:, 7:8]
            bAL = ct[:, 8:9]
            bML = ct[:, 9:10]
            lrA = ct[:, 10:11]  # gamma/beta leaky slope

            def binarize(dst, src):
                if alpha_imm is not None:
                    nc.vector.tensor_scalar(dst, src, alpha_imm, None, Op.is_ge)
                else:
                    # general path: consts col 0 holds -alpha; shift on ACT,
                    # then threshold against 0 with a float immediate
                    nc.scalar.add(src, src, alpha)
                    nc.vector.tensor_scalar(dst, src, 0.0, None, Op.is_ge)

            def binarize_halo_right(dst, src_col):
                # src_col not yet alpha-shifted (main binarize of its tile
                # comes later in program order) -- use a temp, don't mutate
                if alpha_imm is not None:
                    nc.vector.tensor_scalar(dst, src_col, alpha_imm, None, Op.is_ge)
                else:
                    th = fixp.tile([P, 1], F32, tag="bh")
                    nc.scalar.add(th[:], src_col, alpha)
                    nc.vector.tensor_scalar(dst, th[:], 0.0, None, Op.is_ge)

            def binarize_halo_left(dst, src_col):
                # src_col already alpha-shifted by its tile's main binarize
                if alpha_imm is not None:
                    nc.vector.tensor_scalar(dst, src_col, alpha_imm, None, Op.is_ge)
                else:
                    nc.vector.tensor_scalar(dst, src_col, 0.0, None, Op.is_ge)

            def combine(dst, a, m):
                z = zeta_imm if zeta_imm is not None else zeta
                nc.vector.scalar_tensor_tensor(dst, a, z, m, Op.add, Op.subtract)

            # tile list per pass: uniform W-wide tiles, optionally tapering
            # the last lt-tile into MMN-wide mini-tiles to shorten the
            # single-pass serial drain (load->binarize->MM->ACT->combine->
            # store of the final tile runs while DMA engines idle).
            if taper and dt_mult == 1 and not dma_only:
                tiles_per_pass = [(i * lt, lt) for i in range(nt - 1)] + [
                    (l_total - lt + i * MMN, MMN) for i in range(lt // MMN)
                ]
            else:
                tiles_per_pass = [(i * W, W) for i in range(nd)]

            def emit_compute(g_t, s_, w_, first, last):
                o_t = outp.tile([P, w_], ODT, tag="o_t")
                for co in range(0, w_, lt):
                    cw = min(lt, w_ - co)
                    ps = psp.tile([P, cw], F32, tag="ps")
                    for j in range(cw // MMN):
                        for k in range(K):
                            nc.tensor.matmul(
                                ps[:, j * MMN : (j + 1) * MMN],
                                wt[:, k * P : (k + 1) * P],
                                g_t[:, co + j * MMN + k : co + j * MMN + k + MMN],
                                start=(k == 0),
                                stop=(k == K - 1),
                            )

                    if lrelu:
                        # out = Lrelu(scA*ps + bA; slope lrA)
                        #     = beta*max(y,0) + gamma*min(y,0), y = 2ps+b-c
                        ob = o_t[:, co : co + cw]
                        nc.scalar.activation(
                            ob, ps[:], Lrelu, bias=bA, scale=scA, alpha=lrA
                        )
                        if first and co == 0:
                            nc.scalar.activation(
                                o_t[:, 0:1], ps[:, 0:1], Lrelu,
                                bias=bA0, scale=scA, alpha=lrA,
                            )
                        if last and co + cw == w_:
                            nc.scalar.activation(
                                o_t[:, w_ - 1 : w_], ps[:, cw - 1 : cw], Lrelu,
                                bias=bAL, scale=scA, alpha=lrA,
                            )
                        continue

                    a_t = am.tile([P, cw], F32, tag="A")
                    m_t = am.tile([P, cw], F32, tag="M")
                    nc.scalar.activation(a_t[:], ps[:], Relu, bias=bA, scale=scA)
                    nc.scalar.activation(m_t[:], ps[:], Relu, bias=bM, scale=scM)
                    combine(o_t[:, co : co + cw], a_t[:], m_t[:])

                    # boundary columns: missing conv tap -> corrected constants
                    if first and co == 0:
                        fa = fixp.tile([P, 1], F32, tag="fa")
                        fm = fixp.tile([P, 1], F32, tag="fm")
                        nc.scalar.activation(
                            fa[:], ps[:, 0:1], Relu, bias=bA0, scale=scA
                        )
                        nc.scalar.activation(
                            fm[:], ps[:, 0:1], Relu, bias=bM0, scale=scM
                        )
                        combine(o_t[:, 0:1], fa[:], fm[:])
                    if last and co + cw == w_:
                        fa = fixp.tile([P, 1], F32, tag="fa")
                        fm = fixp.tile([P, 1], F32, tag="fm")
                        nc.scalar.activation(
                            fa[:], ps[:, cw - 1 : cw], Relu, bias=bAL, scale=scA
                        )
                        nc.scalar.activation(
                            fm[:], ps[:, cw - 1 : cw], Relu, bias=bML, scale=scM
                        )
                        combine(o_t[:, w_ - 1 : w_], fa[:], fm[:])

                store_dma(o_f[:, s_ : s_ + w_], o_t[:])

            # Aligned loads (exactly w_ cols from s_) measured ~14 us/pass
            # faster than halo loads from s_-1: the 2 halo columns of each
            # g tile are instead binarized from the neighbor x tiles already
            # resident in SBUF.  The right halo needs the NEXT tile's x, so
            # each tile's compute is emitted one iteration later (pend).
            prev_x = None
            prev_w = 0
            pend = None  # (g_t, s_, w_, first) awaiting right halo + compute
            dummy = None
            if dma_only and out_f16:
                dummy = constp.tile([P, W], ODT)
                nc.vector.memset(dummy[:], 0.0)
            for s_, w_ in [tw for _ in range(repeat) for tw in tiles_per_pass]:
                first = s_ == 0
                last = s_ + w_ == l_total
                x_t = xin.tile([P, w_], xdt, tag="x_t")
                if dma_only:
                    # bandwidth-floor ablation: load + store only.
                    # dma_only=2: replicate a misaligned halo load pattern.
                    if dma_only == 2 and not first and not last:
                        xh = xin.tile([P, w_ + 2], xdt, tag="x_t")
                        load_dma(xh[:], x_f[:, s_ - 1 : s_ + w_ + 1])
                        store_dma(o_f[:, s_ : s_ + w_], xh[:, 0:w_])
                    else:
                        load_dma(x_t[:], x_f[:, s_ : s_ + w_])
                        if dummy is not None:
                            store_dma(o_f[:, s_ : s_ + w_], dummy[:, 0:w_])
                        else:
                            store_dma(o_f[:, s_ : s_ + w_], x_t[:])
                    continue
                load_dma(x_t[:], x_f[:, s_ : s_ + w_])
                if pend is not None:
                    pg, ps_, pw_, pfirst = pend
                    plast = ps_ + pw_ == l_total
                    if plast:
                        nc.vector.memset(pg[:, pw_ + 1 : pw_ + 2], 0.0)
                    else:
                        binarize_halo_right(pg[:, pw_ + 1 : pw_ + 2], x_t[:, 0:1])
                    emit_compute(pg, ps_, pw_, pfirst, plast)
                g_t = gbuf.tile([P, w_ + 2], BF16, tag="g_t")
                if first:
                    nc.vector.memset(g_t[:, 0:1], 0.0)
                else:
                    binarize_halo_left(g_t[:, 0:1], prev_x[:, prev_w - 1 : prev_w])
                binarize(g_t[:, 1 : w_ + 1], x_t[:])
                prev_x, prev_w = x_t, w_
                pend = (g_t, s_, w_, first)

            if pend is not None and not dma_only:
                pg, ps_, pw_, pfirst = pend
                nc.vector.memset(pg[:, pw_ + 1 : pw_ + 2], 0.0)
                emit_compute(pg, ps_, pw_, pfirst, True)
    nc.compile()
    return nc


def _prep_params(w, b, alpha, beta, gamma, zeta):
    w = np.asarray(w, np.float32)
    b = np.asarray(b, np.float32).reshape(COUT)
    al = np.asarray(alpha, np.float32).reshape(CIN)
    be = np.asarray(beta, np.float32).reshape(COUT)
    ga = np.asarray(gamma, np.float32).reshape(COUT)
    ze = np.asarray(zeta, np.float32).reshape(COUT)
    assert (be >= 0).all() and (ga >= 0).all(), (
        "kernel assumes beta, gamma >= 0 (RPReLU slopes)"
    )

    wb = np.where(w >= 0, np.float32(1.0), np.float32(-1.0))  # [COUT, CIN, K]
    c = wb.sum(axis=(1, 2), dtype=np.float32)  # interior correction
    c0 = c - wb[:, :, 0].sum(axis=1, dtype=np.float32)  # l = 0 (no left tap)
    cL = c - wb[:, :, K - 1].sum(axis=1, dtype=np.float32)  # l = L-1

    # block-diagonal lhsT per tap: rows (img, cin) -> cols (img, cout)
    wm = np.zeros((P, K * P), np.float32)
    for k in range(K):
        blk = wb[:, :, k].T  # [CIN, COUT]
        for i in range(BPC):
            wm[
                i * CIN : (i + 1) * CIN, k * P + i * COUT : k * P + (i + 1) * COUT
            ] = blk
    wm = wm.astype(ml_dtypes.bfloat16)

    def t2(v):
        return np.tile(np.asarray(v, np.float32), BPC)[:, None]

    lr_slope = np.where(be > 0, ga / np.where(be > 0, be, 1.0), 0.0)
    consts = np.concatenate(
        [
            t2(-al),
            t2(2.0 * be),
            t2(be * (b - c)),
            t2(-2.0 * ga),
            t2(ga * (c - b)),
            t2(ze),
            t2(be * (b - c0)),
            t2(ga * (c0 - b)),
            t2(be * (b - cL)),
            t2(ga * (cL - b)),
            t2(lr_slope),
        ],
        axis=1,
    ).astype(np.float32)
    return wm, consts


def _prep_params_v3(w, b, alpha, beta, gamma, zeta):
    """Host prep for the v3 fp8 path; returns None if preconditions fail
    (then the general fallback path is used)."""
    w = np.asarray(w, np.float32)
    b = np.asarray(b, np.float32).reshape(COUT)
    al = np.asarray(alpha, np.float32).reshape(CIN)
    be = np.asarray(beta, np.float32).reshape(COUT)
    ga = np.asarray(gamma, np.float32).reshape(COUT)
    ze = np.asarray(zeta, np.float32).reshape(COUT)
    if not (np.all(ze == 0.0) and np.all(be > 0.0) and np.all(ga >= 0.0)
            and np.all(be >= ga)):
        return None
    r = (ga / be).astype(np.float32)
    if not np.all(r == r[0]):
        return None  # stt ratio must be a single immediate
    ratio = float(r[0])
    npf8 = mybir.dt.np(FP8)

    wb = np.where(w >= 0, np.float32(1.0), np.float32(-1.0))  # [COUT, CIN, K]
    wsc = be[:, None, None] * wb
    if not np.array_equal(np.asarray(wsc.astype(npf8), np.float32), wsc):
        return None  # beta*(+-1) must be fp8-exact
    bb = (be * b).astype(np.float32)
    if np.max(np.abs(np.asarray(bb.astype(npf8), np.float32) - bb)) > 0.05:
        return None

    wq = np.zeros((P, 2, 2, P), np.float32)
    for i in range(BPC):
        rs, cs = i * CIN, i * COUT
        for k in range(K):
            wq[rs : rs + CIN, k // 2, k % 2, cs : cs + COUT] = wsc[:, :, k].T
        wq[rs, 1, 1, cs : cs + COUT] = bb  # bias tap row (reads ones plane)
    wq = wq.astype(npf8)
    consts = np.tile((np.float32(1e-20) - al), BPC)[:, None].astype(np.float32)
    return wq, consts, ratio


def prep_in_maps(x, w, b, alpha, beta, gamma, zeta):
    """Shared by kernel() and perf.py: per-core input maps + build kwargs."""
    x = np.ascontiguousarray(np.asarray(x), dtype=np.float32)
    assert x.shape == (B, CIN, L)

    v3 = _prep_params_v3(w, b, alpha, beta, gamma, zeta)
    if v3 is not None:
        wq, consts, ratio = v3
        in_maps = [
            {"x": x[i * BPC : (i + 1) * BPC], "wq": wq, "consts": consts}
            for i in range(NCORES)
        ]
        return in_maps, {"v3": True, "ratio": ratio}

    wm, consts = _prep_params(w, b, alpha, beta, gamma, zeta)
    al = np.asarray(alpha, np.float32).ravel()
    ze = np.asarray(zeta, np.float32).ravel()
    alpha_imm = float(al[0]) if np.all(al == al[0]) else None
    zeta_imm = float(ze[0]) if np.all(ze == ze[0]) else None
    in_maps = [
        {"x": x[i * BPC : (i + 1) * BPC], "wmats": wm, "consts": consts}
        for i in range(NCORES)
    ]
    return in_maps, {
        "alpha_imm": alpha_imm,
        "zeta_imm": zeta_imm,
        "lrelu": False,
        "out_f16": True,
    }


def kernel(x, w, b, alpha, beta, gamma, zeta):
    global LAST_RESULTS
    in_maps, build_kw = prep_in_maps(x, w, b, alpha, beta, gamma, zeta)

    key = ("nc",) + tuple(sorted(build_kw.items(), key=str))
    if key not in _NC_CACHE:
        _NC_CACHE[key] = build_nc(**build_kw)
    nc = _NC_CACHE[key]

    res = run_bass_kernel_spmd(
        nc,
        in_maps,
        list(range(NCORES)),
        trace=bool(int(os.environ.get("KERNEL_TRACE", "0"))),
    )
    LAST_RESULTS = res
    out = np.concatenate(
        [np.asarray(res.results[i]["out"]) for i in range(NCORES)], axis=0
    )
    if out.dtype != np.float32:
        out = out.astype(np.float32)
    return out

